# revision 45
# baseline (speedup 1.0000x reference)
"""GAT (3-layer, PyG GATConv-style) Trainium2 Bass kernel, 8-way node-sharded.

v3 design: destination nodes are assigned to (core, block, partition) slots by
global degree-sorted round-robin, so each SBUF partition owns exactly one dst
node and all its incoming edges live in that partition's row:

    G[p, c, :] = h_src of the c-th in-edge of the node in partition p

This makes the whole attention + aggregation pipeline partition-local:
  - per-edge dst attention term  = per-partition broadcast of adall[p] (free)
  - segment softmax denominator  = free-axis reduce of ee
  - weighted message aggregation = the dst incidence matrix is the IDENTITY,
    so the chunk sum runs on the idle TensorEngine as PSUM-accumulated
    ident.T @ M[:,n,:] matmuls (f32-exact, no DVE reduce)

Padding edge slots point at a poison table row whose h solves
<h_p, a_src_head> = -1e12 for every head, so exp(leaky(...)) == 0 exactly and
padding never contributes.  Per-layer halo exchange = AllGather of the bf16 h
table in 2 chunks; phase_a (node transform, feature-major input, PSUM drained
by the Scalar engine) for layer l+1 is emitted interleaved into phase_b of
layer l so the first AllGather overlaps the previous layer's edge compute.
"""

import sys
import numpy as np

sys.path.insert(0, "/opt/trn_rl_repo")

import ml_dtypes  # noqa: E402

BF16 = ml_dtypes.bfloat16

N_CORES = 8
NEG_SLOPE = 0.2
BIG = 1.0e12
GMAX = 3          # chunks per dma_gather; small pieces spread the DMA load
                  # across the 4 SWDGE queues so their rings drain in parallel
NAG = 2           # AllGather chunks per layer
SBF = 15          # phase_a blocks per chunk (NBLK must be NAG*SBF)


def _wrap16(idx_list):
    """Pack an int16 index list into the [128, n/16] wrapped+replicated SBUF
    layout dma_gather expects (index i -> partition i%16, col i//16; the 16-row
    block replicated to all 8 gpsimd core groups)."""
    n = len(idx_list)
    assert n % 16 == 0
    w = np.asarray(idx_list, np.int16).reshape(n // 16, 16).T  # [16, n/16]
    return np.tile(w, (8, 1))  # [128, n/16]


def _preprocess(x, edge_index, n_cores, N, D):
    """CPU-side: add self loops, assign dst nodes to (core, slot) by global
    degree-sorted round-robin, build the per-block dst-partition gather index
    tables, and the static per-block chunk schedule."""
    NPC = N // n_cores
    NBLK = (NPC + 127) // 128
    NPC_PAD = NBLK * 128

    src = np.concatenate([edge_index[0].astype(np.int64), np.arange(N, dtype=np.int64)])
    dst = np.concatenate([edge_index[1].astype(np.int64), np.arange(N, dtype=np.int64)])

    deg = np.bincount(dst, minlength=N)
    order = np.argsort(-deg, kind="stable")          # nodes by degree desc
    node_core = np.empty(N, np.int64)
    node_slot = np.empty(N, np.int64)
    node_core[order] = np.arange(N) % n_cores
    node_slot[order] = np.arange(N) // n_cores
    # global (all-gathered, core-major) table row of a node
    node_row = node_core * NPC_PAD + node_slot

    # static per-block chunk schedule = max in-degree in that block, over cores
    deg_sorted = deg[order]
    sched = np.zeros(NBLK, np.int64)
    for k in range(n_cores):
        dk = deg_sorted[k::n_cores]
        for b in range(NBLK):
            blk = dk[b * 128:(b + 1) * 128]
            if len(blk):
                sched[b] = max(sched[b], blk.max())
    sched = np.maximum(sched, 1)
    NCHS = int(sched.sum())

    # route edges: per core, per block, index position c*128 + p = c-th edge
    # of the node in partition p; pad with the core's poison row.
    d_core = node_core[dst]
    d_slot = node_slot[dst]
    s_row = node_row[src]
    srcw = np.zeros((n_cores, 128, NCHS * 8), np.int16)
    coff = np.concatenate([[0], np.cumsum(sched)])   # chunk offset per block
    for k in range(n_cores):
        poison = k * NPC_PAD + (NPC_PAD - 1)
        mask = d_core == k
        sl = d_slot[mask]
        sr = s_row[mask]
        o2 = np.argsort(sl, kind="stable")
        sl, sr = sl[o2], sr[o2]
        starts = np.searchsorted(sl, np.arange(NPC))
        cidx = np.arange(len(sl)) - starts[sl]       # within-node edge counter
        blk = sl // 128
        p = sl % 128
        idx = np.full(NCHS * 128, poison, np.int64)
        pos = (coff[blk] + cidx) * 128 + p
        idx[pos] = sr
        for b in range(NBLK):
            seg = idx[coff[b] * 128:coff[b + 1] * 128]
            srcw[k, :, coff[b] * 8:coff[b + 1] * 8] = _wrap16(seg)

    # per-core input x, slot order, feature-major (transposed)
    x_shT = np.zeros((n_cores, D, NPC_PAD), np.float32)
    xv = np.asarray(x, np.float32)
    x_shT[node_core, :, node_slot] = xv

    return dict(
        NPC=NPC, NBLK=NBLK, NPC_PAD=NPC_PAD, sched=sched.tolist(),
        srcw=srcw, x_shT=x_shT, node_core=node_core, node_slot=node_slot,
    )


def _build_program(N, D, H, C, NBLK, sched, n_cores):
    """Emit the full 3-layer Bass/Tile program (SPMD, identical per core)."""
    from concourse import bacc, tile, mybir

    HC = H * C
    NPC_PAD = NBLK * 128
    OUTC = HC + 4          # node matmul out cols: h | al_dst
    NCHS = sum(sched)
    # blocks are processed in fused pairs: big DVE ops span both blocks
    NPAIR = NBLK // 2
    NCHP_MAX = max(sched[2 * p] + sched[2 * p + 1] for p in range(NPAIR))
    coff = [0]
    for s in sched:
        coff.append(coff[-1] + s)
    CHR = NPC_PAD // NAG             # rows per AllGather chunk
    assert NBLK == NAG * SBF
    f32 = mybir.dt.float32
    bf16 = mybir.dt.bfloat16
    i16 = mybir.dt.int16
    AF = mybir.ActivationFunctionType
    ALU = mybir.AluOpType

    nc = bacc.Bacc("TRN2", target_bir_lowering=False, debug=False,
                   num_devices=n_cores, num_swdge_queues=4)

    # -------- I/O --------
    xT_in = nc.dram_tensor("x_shT", [D, NPC_PAD], f32, kind="ExternalInput").ap()
    srcw_in = nc.dram_tensor("srcw", [128, NCHS * 8], i16, kind="ExternalInput").ap()
    ident_in = nc.dram_tensor("ident", [128, 128], f32, kind="ExternalInput").ap()
    identb_in = nc.dram_tensor("identb", [128, 128], bf16, kind="ExternalInput").ap()
    wext_in = [nc.dram_tensor(f"wext{l}", [128, OUTC], f32, kind="ExternalInput").ap()
               for l in range(3)]
    asrx_in = [nc.dram_tensor(f"asrx{l}", [128, NCHP_MAX * HC], bf16,
                              kind="ExternalInput").ap() for l in range(3)]
    pois_in = [nc.dram_tensor(f"pois{l}", [1, HC], bf16, kind="ExternalInput").ap()
               for l in range(3)]
    bias_in = [nc.dram_tensor(f"bias{l}", [128, HC], f32, kind="ExternalInput").ap()
               for l in range(3)]
    out_sh = nc.dram_tensor("out_sh", [NPC_PAD, 2 * HC], f32, kind="ExternalOutput").ap()

    with tile.TileContext(nc) as tc:
        with tc.tile_pool(name="dram", bufs=1, space="DRAM") as dram, \
             tc.tile_pool(name="res", bufs=1) as res, \
             tc.tile_pool(name="gath", bufs=3) as gath, \
             tc.tile_pool(name="work", bufs=2) as work, \
             tc.tile_pool(name="small", bufs=3) as small, \
             tc.tile_pool(name="psA", bufs=2, space="PSUM") as psA, \
             tc.tile_pool(name="psB", bufs=3, space="PSUM") as psB:

            # ---- internal DRAM ----
            yh_in = [dram.tile([NPC_PAD, HC], bf16, name=f"yh_in{l}")
                     for l in range(3)]
            YHG = [[dram.tile([CHR * n_cores, HC], bf16, addr_space="Shared",
                              name=f"YHG{l}_{j}") for j in range(NAG)]
                   for l in range(3)]
            YHL = [dram.tile([n_cores * NPC_PAD, HC], bf16, name=f"YHL{l}")
                   for l in range(3)]
            XTA = dram.tile([D, NPC_PAD], f32)
            XTB = dram.tile([D, NPC_PAD], f32)

            # ---- resident constants / tables ----
            srcw_sb = res.tile([128, NCHS * 8], i16)
            nc.sync.dma_start(srcw_sb[:], srcw_in)
            ident_sb = res.tile([128, 128], f32)
            nc.sync.dma_start(ident_sb[:], ident_in)
            identb_sb = res.tile([128, 128], bf16)
            nc.sync.dma_start(identb_sb[:], identb_in)
            wext_sb, asrx_sb, pois_sb, bias_sb, adall = [], [], [], [], []
            for l in range(3):
                w = res.tile([128, OUTC], f32, name=f"wext_sb{l}")
                nc.sync.dma_start(w[:], wext_in[l])
                wext_sb.append(w)
                a = res.tile([128, NCHP_MAX, HC], bf16, name=f"asrx_sb{l}")
                nc.sync.dma_start(a[:].rearrange("p n c -> p (n c)"), asrx_in[l])
                asrx_sb.append(a)
                pz = res.tile([1, HC], bf16, name=f"pois_sb{l}")
                nc.sync.dma_start(pz[:], pois_in[l])
                pois_sb.append(pz)
                bt = res.tile([128, HC], f32, name=f"bias_sb{l}")
                nc.sync.dma_start(bt[:], bias_in[l])
                bias_sb.append(bt)
                adall.append(res.tile([128, NBLK, H], f32, name=f"adall{l}"))

            def phase_a_half(l, x_srcT, hj):
                """Transform SBF blocks of nodes, store the bf16 h rows, then
                AllGather this chunk and re-layout it into the local table."""
                cols = slice(hj * CHR, (hj + 1) * CHR)
                xt = work.tile([128, CHR], f32, tag="xt")
                nc.sync.dma_start(xt[:], x_srcT[:, cols])
                slab = work.tile([128, SBF, HC], bf16, tag="slab")
                for i in range(SBF):
                    b = hj * SBF + i
                    ya = psA.tile([128, OUTC], f32, tag="ya")
                    nc.tensor.matmul(ya[:], xt[:, i * 128:(i + 1) * 128],
                                     wext_sb[l][:], start=True, stop=True)
                    nc.scalar.activation(slab[:, i, :], ya[:, 0:HC], AF.Copy)
                    nc.scalar.activation(adall[l][:, b, :], ya[:, HC:OUTC],
                                         AF.Copy)
                nc.sync.dma_start(
                    yh_in[l][cols, :].rearrange("(i p) c -> p i c", p=128),
                    slab[:])
                if hj == NAG - 1:
                    # overwrite the last pad row with the poison h vector
                    nc.sync.dma_start(yh_in[l][NPC_PAD - 1:NPC_PAD, :],
                                      pois_sb[l][:])
                nc.gpsimd.collective_compute(
                    "AllGather", mybir.AluOpType.bypass,
                    replica_groups=[list(range(n_cores))],
                    ins=[yh_in[l][cols, :].opt()],
                    outs=[YHG[l][hj][:].opt()],
                )
                nc.sync.dma_start(
                    YHL[l][:].rearrange("(k s) c -> k s c",
                                        k=n_cores)[:, cols, :],
                    YHG[l][hj][:].rearrange("(k r) c -> k r c", k=n_cores))

            qn = [0]

            def phase_b_front(l, pr):
                """Gather + per-edge attention + messages for a fused pair of
                blocks (big DVE ops span both); kick off the per-block PE
                chunk-sums.  Returns [(b, acc, rd), ...] for the lagged
                epilogues."""
                b0, b1 = 2 * pr, 2 * pr + 1
                n0, n1 = sched[b0], sched[b1]
                NP = n0 + n1
                G = gath.tile([128, NCHP_MAX, HC], bf16, tag="G")
                for g0 in range(0, NP, GMAX):
                    gn = min(GMAX, NP - g0)
                    ic = slice((coff[b0] + g0) * 8, (coff[b0] + g0 + gn) * 8)
                    nc.gpsimd.dma_gather(G[:, g0:g0 + gn, :], YHL[l][:],
                                         srcw_sb[:, ic], num_idxs=gn * 128,
                                         num_idxs_reg=gn * 128, elem_size=HC,
                                         queue_num=qn[0] % 4)
                    qn[0] += 1
                G4 = G[:, 0:NP, :].rearrange("p n (h c) -> p n h c", h=H)
                # per-edge src attention logits: sum over C per head
                alprod = work.tile([128, NCHP_MAX, HC], bf16, tag="alprod")
                nc.vector.tensor_tensor(alprod[:, 0:NP, :], G[:, 0:NP, :],
                                        asrx_sb[l][:, 0:NP, :], ALU.mult)
                als = small.tile([128, H, NCHP_MAX], f32, tag="als")
                nc.vector.tensor_reduce(
                    als[:, :, 0:NP].rearrange("p h n -> p n h"),
                    alprod[:, 0:NP, :].rearrange("p n (h c) -> p n h c", h=H),
                    mybir.AxisListType.X, ALU.add)
                # + per-partition dst term (per sub-block), leaky_relu, exp
                alsum = small.tile([128, H, NCHP_MAX], f32, tag="alsum")
                for b, sl in ((b0, slice(0, n0)), (b1, slice(n0, NP))):
                    nc.vector.tensor_tensor(
                        alsum[:, :, sl], als[:, :, sl],
                        adall[l][:, b, :].unsqueeze(2)
                        .broadcast_to([128, H, sl.stop - sl.start]),
                        ALU.add)
                smul = small.tile([128, H, NCHP_MAX], f32, tag="smul")
                nc.vector.tensor_scalar_mul(smul[:, :, 0:NP],
                                            alsum[:, :, 0:NP], NEG_SLOPE)
                lk = small.tile([128, H, NCHP_MAX], f32, tag="lk")
                nc.vector.tensor_tensor(lk[:, :, 0:NP], alsum[:, :, 0:NP],
                                        smul[:, :, 0:NP], ALU.max)
                ee = small.tile([128, H, NCHP_MAX], bf16, tag="ee")
                nc.scalar.activation(ee[:, :, 0:NP], lk[:, :, 0:NP], AF.Exp)
                # messages in natural chunk-major layout (fast contiguous mult)
                Mt = work.tile([128, NCHP_MAX, HC], bf16, tag="Mt")
                nc.vector.tensor_tensor(
                    Mt[:, 0:NP, :].rearrange("p n (h c) -> p n h c", h=H),
                    G4,
                    ee[:, :, 0:NP].rearrange("p h n -> p n h")
                    .unsqueeze(3).broadcast_to([128, NP, H, C]),
                    ALU.mult)
                out = []
                for b, sl in ((b0, slice(0, n0)), (b1, slice(n0, NP))):
                    den = small.tile([128, H], f32, tag="den")
                    nc.vector.tensor_reduce(den[:], ee[:, :, sl],
                                            mybir.AxisListType.X, ALU.add)
                    dn = small.tile([128, H], f32, tag="dn")
                    nc.vector.tensor_scalar_add(dn[:], den[:], 1e-16)
                    rd = small.tile([128, H], f32, tag="rd")
                    nc.vector.reciprocal(rd[:], dn[:])
                    # chunk sum on the TensorEngine: dst incidence == identity,
                    # PSUM-accumulated (f32 exact)
                    acc = psB.tile([128, HC], f32, tag="acc")
                    for j, n in enumerate(range(sl.start, sl.stop)):
                        nc.tensor.matmul(acc[:], identb_sb[:], Mt[:, n, :],
                                         start=(j == 0),
                                         stop=(n == sl.stop - 1))
                    out.append((b, acc, rd))
                return out

            def phase_b_epilogue(l, b, acc, rd, x_dstT, last):
                rows = slice(b * 128, (b + 1) * 128)
                o = work.tile([128, HC], f32, tag="o")
                nc.vector.tensor_tensor(
                    o[:].rearrange("p (h c) -> p h c", h=H),
                    acc[:].rearrange("p (h c) -> p h c", h=H),
                    rd[:].unsqueeze(2).broadcast_to([128, H, C]), ALU.mult)
                ob = work.tile([128, HC], f32, tag="ob")
                nc.vector.tensor_add(ob[:], o[:], bias_sb[l][:])
                if not last:
                    # L2 normalize rows + relu; write transposed into the
                    # resident next-layer input
                    sq = work.tile([128, HC], f32, tag="sq")
                    nc.vector.tensor_tensor(sq[:], ob[:], ob[:], ALU.mult)
                    ss = small.tile([128, 1], f32, tag="ss")
                    nc.vector.tensor_reduce(ss[:], sq[:],
                                            mybir.AxisListType.X, ALU.add)
                    ssc = small.tile([128, 1], f32, tag="ssc")
                    nc.vector.tensor_scalar_max(ssc[:], ss[:], 1e-24)
                    sr = small.tile([128, 1], f32, tag="sr")
                    nc.scalar.activation(sr[:], ssc[:], AF.Sqrt)
                    rn = small.tile([128, 1], f32, tag="rn")
                    nc.vector.reciprocal(rn[:], sr[:])
                    xr = work.tile([128, HC], f32, tag="xr")
                    nc.vector.tensor_scalar(xr[:], ob[:], rn[:], 0.0,
                                            ALU.mult, ALU.max)
                    xrt_ps = psB.tile([128, HC], f32, tag="xrt_ps")
                    nc.tensor.transpose(xrt_ps[:], xr[:], ident_sb[:])
                    xrt = work.tile([128, HC], f32, tag="xrt")
                    nc.scalar.activation(xrt[:], xrt_ps[:], AF.Copy)
                    nc.sync.dma_start(x_dstT[:, rows], xrt[:])
                else:
                    nc.sync.dma_start(out_sh[rows, 0:HC], ob[:])
                    m = small.tile([128, 1], f32, tag="m")
                    nc.vector.tensor_reduce(m[:], ob[:],
                                            mybir.AxisListType.X, ALU.max)
                    negm = small.tile([128, 1], f32, tag="negm")
                    nc.vector.tensor_scalar_mul(negm[:], m[:], -1.0)
                    ex = work.tile([128, HC], f32, tag="ex")
                    nc.scalar.activation(ex[:], ob[:], AF.Exp,
                                         bias=negm[:])
                    se = small.tile([128, 1], f32, tag="se")
                    nc.vector.tensor_reduce(se[:], ex[:],
                                            mybir.AxisListType.X, ALU.add)
                    rs = small.tile([128, 1], f32, tag="rs")
                    nc.vector.reciprocal(rs[:], se[:])
                    pr = work.tile([128, HC], f32, tag="pr")
                    nc.vector.tensor_tensor(
                        pr[:], ex[:], rs[:].broadcast_to([128, HC]),
                        ALU.mult)
                    nc.sync.dma_start(out_sh[rows, HC:2 * HC], pr[:])

            # emission: phase_a(l+1) halves interleave into phase_b(l) so the
            # first AllGather overlaps the previous layer's edge compute
            xsrc = [xT_in, XTA[:], XTB[:]]
            xdst = [XTA[:], XTB[:], None]
            phase_a_half(0, xsrc[0], 0)
            phase_a_half(0, xsrc[0], 1)
            for l in range(3):
                last = l == 2
                pend = []
                for pr in range(NPAIR):
                    st = phase_b_front(l, pr)
                    for (b, acc, rd) in pend:
                        phase_b_epilogue(l, b, acc, rd, xdst[l], last)
                    pend = st
                    if not last and pr == SBF // 2 + 1:
                        # epilogues 0..SBF-1 are done -> first half of the
                        # next layer's node transform + AllGather can start
                        phase_a_half(l + 1, xsrc[l + 1], 0)
                for (b, acc, rd) in pend:
                    phase_b_epilogue(l, b, acc, rd, xdst[l], last)
                if not last:
                    phase_a_half(l + 1, xsrc[l + 1], 1)

    nc.compile()
    return nc


def _make_in_maps(inputs, pre, N, D, H, C, n_cores):
    HC = H * C
    sched = pre["sched"]
    NCH_MAX = max(sched[2 * p] + sched[2 * p + 1]
                  for p in range(len(sched) // 2))
    wext, asrx, pois, bias = [], [], [], []
    for l in range(3):
        W = np.asarray(inputs[f"W{l}"], np.float32)
        a_s = np.asarray(inputs[f"a_src{l}"], np.float32)
        a_d = np.asarray(inputs[f"a_dst{l}"], np.float32)
        b = np.asarray(inputs[f"b{l}"], np.float32)
        wtil = np.stack([W[:, h * C:(h + 1) * C] @ a_d[h] for h in range(H)], axis=1)
        wext.append(np.concatenate([W, wtil], axis=1))          # [D, HC+4]
        asrx.append(np.tile(a_s.reshape(1, HC), (128, NCH_MAX)).astype(BF16))
        hp = np.concatenate([a_s[h] * (-BIG / (a_s[h] @ a_s[h]))
                             for h in range(H)])
        pois.append(hp.reshape(1, HC).astype(BF16))
        bias.append(np.tile(b.reshape(1, HC), (128, 1)))
    ident = np.eye(128, dtype=np.float32)

    in_maps = []
    for k in range(n_cores):
        m = {
            "x_shT": pre["x_shT"][k],
            "srcw": pre["srcw"][k],
            "ident": ident,
            "identb": ident.astype(BF16),
        }
        for l in range(3):
            m[f"wext{l}"] = wext[l]
            m[f"asrx{l}"] = asrx[l]
            m[f"pois{l}"] = pois[l]
            m[f"bias{l}"] = bias[l]
        in_maps.append(m)
    return in_maps


def _run(inputs, N, D, H, C, n_cores=N_CORES, trace=False):
    from concourse import bass_utils

    HC = H * C
    x = np.asarray(inputs["x"], np.float32)
    edge_index = np.asarray(inputs["edge_index"])
    pre = _preprocess(x, edge_index, n_cores, N, D)
    NBLK = pre["NBLK"]

    nc = _build_program(N, D, H, C, NBLK, pre["sched"], n_cores)
    in_maps = _make_in_maps(inputs, pre, N, D, H, C, n_cores)

    res = bass_utils.run_bass_kernel_spmd(
        nc, in_maps, core_ids=list(range(n_cores)), trace=trace)

    outs = np.stack([res.results[k]["out_sh"] for k in range(n_cores)])
    full = outs[pre["node_core"], pre["node_slot"], :]       # [N, 2*HC]
    logits = np.ascontiguousarray(full[:, 0:HC])
    probs = np.ascontiguousarray(full[:, HC:2 * HC])
    return (logits, probs), res


def kernel(**inputs):
    (logits, probs), _ = _run(inputs, N=30000, D=128, H=4, C=32)
    return (logits, probs)
